# revision 3
# baseline (speedup 1.0000x reference)
"""Trainium2 Bass kernel v2 for nn_GCNDeno (per-sample cosine-graph ChebConv).

Data-parallel over B=128 graphs: 16 graphs per core. Per graph:
  - gather app rows (bf16), emit out_app (bf16, host widens)
  - normalize rows -> F (bf16, x16 scale), PE-transpose -> Ft (fp8 e4m3)
  - Gram G = Ft^T Ft via fp8 DoubleRow matmuls (G = 256*A, PSUM quarters)
  - mean of strict upper via PE matvecs (s rides the Ft-copy accums)
  - adj = (G >= mean)*noeye as FP8 (binary, exact in fp8), deg accum
  - ChebConv with dinv folded into broadcast map tiles:
      x' = 64*dinv.x (fp8); t1 = (-dinv/64).(adj@x'); t1' = -4*dinv^2 . t1;
      y' = (-dinv/64).(adj@t1') = 4*S t1; h = u0^T xT + u1^T t1T + u2^T y'T
    with u = [w0-w2, w1, w2/2] (Chebyshev t2-fold + /4 for the y' scale)
  - relu+bias on Act; layer 2 same; out_x = x2^T (bf16, host transposes)
"""
import numpy as np
import ml_dtypes

import concourse.bass as bass
import concourse.tile as tile
from concourse import bacc, mybir
from concourse.bass_utils import run_bass_kernel_spmd
from concourse.masks import make_identity

B, N, D, R = 128, 512, 128, 512
NODE_VOCAB, APP_VOCAB = 30000, 5000
N_CORES = 8
BL = B // N_CORES
EPS = 1e-12
CNT_UP = N * (N - 1) // 2
P = 128
NB = N // P
KC = R // P

F32 = mybir.dt.float32
F32R = mybir.dt.float32r
BF16 = mybir.dt.bfloat16
FP8 = mybir.dt.float8e4
I32 = mybir.dt.int32
Alu = mybir.AluOpType
Act = mybir.ActivationFunctionType
AX = mybir.AxisListType
PM = mybir.MatmulPerfMode


class _C:
    pass


def _consts(nc, const, dram):
    c = _C()
    identf = const.tile([P, P], F32)
    make_identity(nc, identf[:])
    c.identb = const.tile([P, P], BF16)
    nc.vector.tensor_copy(c.identb[:], identf[:])
    c.ones_b = const.tile([P, P], BF16)
    nc.vector.memset(c.ones_b[:], 1.0)
    c.ones_row = const.tile([1, P], BF16)
    nc.vector.memset(c.ones_row[:], 1.0)
    c.sum_sq = const.tile([P, P], F32)
    nc.vector.memset(c.sum_sq[:], 1.0 / (2.0 * CNT_UP))
    c.eps = const.tile([P, 1], F32)
    nc.vector.memset(c.eps[:], EPS)
    c.noeye = const.tile([P, NB, N], BF16)
    nc.sync.dma_start(c.noeye[:], dram["noeyed"][:])
    for w in ("u1", "u2"):
        t = const.tile([P, 3, P], BF16, tag=w)
        nc.sync.dma_start(t[:], dram[w + "d"][:])
        setattr(c, w, t)
    for bn in ("b1", "b2"):
        t = const.tile([P, 1], F32, tag=bn)
        nc.sync.dma_start(t[:], dram[bn + "d"][:])
        setattr(c, bn, t)
    return c


def _stage_a(nc, pools, c, g, dram, st):
    """Gathers + app output."""
    io, io2, work, small, psG, ps_tp, ps_mm, ps_sm = pools
    idxn = small.tile([P, NB], I32, tag="idxn")
    idxa = small.tile([P, KC], I32, tag="idxa")
    nc.sync.dma_start(idxn[:], dram["idxn"][g])
    nc.sync.dma_start(idxa[:], dram["idxa"][g])

    E = io.tile([P, NB, R], BF16, tag="E")
    for nb in range(NB):
        nc.gpsimd.indirect_dma_start(
            out=E[:, nb, :], out_offset=None, in_=dram["rec"][:],
            in_offset=bass.IndirectOffsetOnAxis(ap=idxa[:, nb:nb + 1], axis=0))
    X = io2.tile([P, NB, D], BF16, tag="X")
    for nb in range(NB):
        nc.gpsimd.indirect_dma_start(
            out=X[:, nb, :], out_offset=None, in_=dram["embed"][:],
            in_offset=bass.IndirectOffsetOnAxis(ap=idxn[:, nb:nb + 1], axis=0))
    nc.scalar.dma_start(dram["out_app"][g], E[:])
    st["E"], st["X"] = E, X


def _stage_b1(nc, pools, c, g, dram, st):
    """Normalize -> F; transpose -> Ft fp8."""
    io, io2, work, small, psG, ps_tp, ps_mm, ps_sm = pools
    E = st["E"]

    ss = small.tile([P, NB], F32, tag="ss")
    dump = small.tile([P, 2, R], BF16, tag="dump")
    for nb in range(NB):
        if nb % 2 == 0:
            nc.vector.scalar_tensor_tensor(
                out=dump[:, 0, :], in0=E[:, nb, :], scalar=1.0,
                in1=E[:, nb, :], op0=Alu.mult, op1=Alu.mult,
                accum_out=ss[:, nb:nb + 1])
        else:
            nc.scalar.activation(dump[:, 1, :], E[:, nb, :], Act.Square,
                                 accum_out=ss[:, nb:nb + 1])
    nrm16 = small.tile([P, NB], F32, tag="nrm16")
    nc.scalar.activation(nrm16[:], ss[:], Act.Sqrt, scale=1.0 / 256.0)
    inv16 = small.tile([P, NB], F32, tag="inv16")
    nc.vector.reciprocal(inv16[:], nrm16[:])
    yield

    F = io.tile([P, NB, R], BF16, tag="F")
    for nb in range(NB):
        nc.vector.tensor_scalar(F[:, nb, :], E[:, nb, :],
                                inv16[:, nb:nb + 1], 0.0,
                                op0=Alu.mult, op1=Alu.add)
    yield

    Ft = io2.tile([P, KC, N], FP8, tag="Ft")
    for kc in range(KC):
        tp = ps_tp.tile([P, N], BF16, tag="tp", space="PSUM")
        for nb in range(NB):
            nc.tensor.transpose(tp[:, nb * P:(nb + 1) * P],
                                F[:, nb, kc * P:(kc + 1) * P],
                                identity=c.identb[:])
        if kc % 2 == 0:
            nc.vector.tensor_scalar(Ft[:, kc, :], tp[:], 1.0, 0.0,
                                    op0=Alu.mult, op1=Alu.add)
        else:
            nc.scalar.copy(Ft[:, kc, :], tp[:])
        if kc == 1:
            yield
    st["Ft"] = Ft


def _stage_b2a(nc, pools, c, g, dram, st):
    """Gram quarters -> As (bf16 SBUF) with rowsum accums -> mean."""
    io, io2, work, small, psG, ps_tp, ps_mm, ps_sm = pools
    Ft = st["Ft"]

    # Gram quarters -> As bf16 (SBUF), noeye folded (zero diagonal);
    # v5 off-diagonal rowsums ride the copies
    As = io2.tile([P, NB, N], BF16, tag="As")
    v5 = small.tile([P, NB], F32, tag="v5")
    gc_eng = [nc.vector, nc.vector, nc.vector, nc.vector]
    for nb in range(NB):
        gq = psG.tile([P, N], F32, tag="G", space="PSUM")
        for k in range(2):
            nc.tensor.matmul(gq[:],
                             lhsT=Ft[:, 2 * k:2 * k + 2, nb * P:(nb + 1) * P],
                             rhs=Ft[:, 2 * k:2 * k + 2, :],
                             start=(k == 0), stop=(k == 1),
                             perf_mode=PM.DoubleRow)
        gc_eng[nb].scalar_tensor_tensor(
            out=As[:, nb, :], in0=gq[:], scalar=1.0,
            in1=c.noeye[:, nb, :], op0=Alu.mult, op1=Alu.mult,
            accum_out=v5[:, nb:nb + 1])
        if nb == 1:
            yield
    mps = ps_sm.tile([P, 8], F32, tag="sm", space="PSUM")
    nc.tensor.matmul(mps[:, 0:NB], lhsT=c.sum_sq[:], rhs=v5[:],
                     start=True, stop=True)
    mean_bc = small.tile([P, 1], F32, tag="mean")
    nc.vector.tensor_reduce(mean_bc[:], mps[:, 0:NB], axis=AX.X, op=Alu.add)
    if g == 0:
        nc.sync.dma_start(dram["dbg_mean"][:], mean_bc[:])
        nc.sync.dma_start(dram["dbg_v5"][:], v5[:])
        nc.sync.dma_start(dram["dbg_as"][:], As[:, 0, :])
    st.update(As=As, mean=mean_bc)


def _stage_b2b(nc, pools, c, g, dram, st):
    """Adjacency (bf16), dinv, S8 (fp8, x256), x8, xT."""
    io, io2, work, small, psG, ps_tp, ps_mm, ps_sm = pools
    X, As, mean_bc = st["X"], st["As"], st["mean"]

    adj = io2.tile([P, NB, N], BF16, tag="adj")
    deg = small.tile([P, NB], F32, tag="deg")
    for nb in range(NB):
        nc.vector.scalar_tensor_tensor(
            out=adj[:, nb, :], in0=As[:, nb, :], scalar=mean_bc[:, :1],
            in1=c.noeye[:, nb, :], op0=Alu.is_ge, op1=Alu.mult,
            accum_out=deg[:, nb:nb + 1])
    yield

    # dinv chain; dinv16 = 16/sqrt(deg)
    nrmd = small.tile([P, NB], F32, tag="nrmd")
    nc.scalar.activation(nrmd[:], deg[:], Act.Sqrt, bias=c.eps[:, :1],
                         scale=1.0 / 256.0)
    dinv16 = small.tile([P, NB], F32, tag="dinv16")
    nc.vector.reciprocal(dinv16[:], nrmd[:])
    # ndbc16[?, m] = 16*dinv_m broadcast across partitions
    dT = ps_tp.tile([P, N], BF16, tag="tp", space="PSUM")
    db = small.tile([P, NB], BF16, tag="dinvb")
    nc.vector.tensor_copy(db[:], dinv16[:])
    for nb in range(NB):
        nc.tensor.transpose(dT[0:1, nb * P:(nb + 1) * P], db[:, nb:nb + 1],
                            identity=c.identb[:])
    dTb = small.tile([1, N], BF16, tag="dTb")
    nc.vector.tensor_copy(dTb[:], dT[0:1, 0:N])
    nd_ps = psG.tile([P, N], F32, tag="G", space="PSUM")
    nc.tensor.matmul(nd_ps[:], lhsT=c.ones_row[:], rhs=dTb[:],
                     start=True, stop=True)
    ndbc = small.tile([P, N], BF16, tag="ndbc")
    nc.scalar.copy(ndbc[:], nd_ps[:])
    yield

    # S8 = -(16 dinv_n) * adj * (16 dinv_m) / 256 scale-carried (fp8)
    nds = small.tile([P, NB], F32, tag="nds")
    nc.vector.tensor_scalar(nds[:], dinv16[:], -1.0, 0.0, op0=Alu.mult, op1=Alu.add)
    S8 = io2.tile([P, NB, N], FP8, tag="S8")
    for nb in range(NB):
        nc.vector.scalar_tensor_tensor(
            out=S8[:, nb, :], in0=adj[:, nb, :], scalar=nds[:, nb:nb + 1],
            in1=ndbc[:], op0=Alu.mult, op1=Alu.mult)
    yield

    # x8 = 64*x (fp8), xT (bf16)
    x8 = work.tile([P, NB, D], FP8, tag="x8")
    nc.scalar.activation(x8[:], X[:], Act.Copy, scale=64.0)
    tpx = ps_tp.tile([P, N], BF16, tag="tp", space="PSUM")
    for nb in range(NB):
        nc.tensor.transpose(tpx[:, nb * P:(nb + 1) * P], X[:, nb, :],
                            identity=c.identb[:])
    xT = work.tile([P, N], BF16, tag="xT")
    nc.scalar.copy(xT[:], tpx[:])
    if g == 0:
        nc.sync.dma_start(dram["dbg_deg"][:], deg[:])
        nc.sync.dma_start(dram["dbg_adj"][:], adj[:])
    st.update(x8=x8, xT=xT, S8=S8)


def _stage_c(nc, pools, c, g, dram, st):
    """Two ChebConv layers, transposed-out applies vs S8."""
    io, io2, work, small, psG, ps_tp, ps_mm, ps_sm = pools
    S8 = st["S8"]
    US = 1.0 / (256.0 * 64.0)    # un-scale for S8(x256) @ (x64) operands

    tin8, t0T = st["x8"], st["xT"]
    for layer, (u, b) in enumerate(((c.u1, c.b1), (c.u2, c.b2))):
        last = layer == 1
        # t1^T = (S t0)^T : [d, n] psum
        p1 = ps_mm.tile([P, N], F32, tag="mm", space="PSUM")
        for k in range(2):
            nc.tensor.matmul(p1[:], lhsT=tin8[:, 2 * k:2 * k + 2, :],
                             rhs=S8[:, 2 * k:2 * k + 2, :],
                             start=(k == 0), stop=(k == 1),
                             perf_mode=PM.DoubleRow)
        yield
        t1T = work.tile([P, N], BF16, tag=f"t1T_{layer}")
        nc.scalar.activation(t1T[:], p1[:], Act.Copy, scale=US)
        if g == 0 and layer == 0:
            nc.sync.dma_start(dram["dbg_t1T"][:], t1T[:])
        tpn = ps_tp.tile([P, N], BF16, tag="tp", space="PSUM")
        for nb in range(NB):
            nc.tensor.transpose(tpn[:, nb * P:(nb + 1) * P],
                                t1T[:, nb * P:(nb + 1) * P],
                                identity=c.identb[:])
        t1n8 = work.tile([P, NB, D], FP8, tag=f"t1n8_{layer}")
        nc.scalar.activation(t1n8[:], tpn[:], Act.Copy, scale=64.0)
        yield
        # y^T = (S t1)^T
        p2 = ps_mm.tile([P, N], F32, tag="mm", space="PSUM")
        for k in range(2):
            nc.tensor.matmul(p2[:], lhsT=t1n8[:, 2 * k:2 * k + 2, :],
                             rhs=S8[:, 2 * k:2 * k + 2, :],
                             start=(k == 0), stop=(k == 1),
                             perf_mode=PM.DoubleRow)
        yT = work.tile([P, N], BF16, tag=f"yT_{layer}")
        if layer == 0:
            nc.scalar.activation(yT[:], p2[:], Act.Copy, scale=US)
            if g == 0:
                nc.sync.dma_start(dram["dbg_yT"][:], yT[:])
        else:
            nc.vector.tensor_scalar(yT[:], p2[:], US, 0.0,
                                    op0=Alu.mult, op1=Alu.add)
        yield
        # h^T = u0^T t0T + u1^T t1T + u2^T yT (+b, relu)
        hp = ps_mm.tile([P, N], F32, tag="mm", space="PSUM")
        nc.tensor.matmul(hp[:], lhsT=u[:, 0, :], rhs=t0T[:], start=True, stop=False)
        nc.tensor.matmul(hp[:], lhsT=u[:, 1, :], rhs=t1T[:], start=False, stop=False)
        nc.tensor.matmul(hp[:], lhsT=u[:, 2, :], rhs=yT[:], start=False, stop=True)
        nxT = work.tile([P, N], BF16, tag=f"nxT_{layer}")
        nc.scalar.activation(nxT[:], hp[:], Act.Relu, bias=b[:, :1])
        if not last:
            tpr = ps_tp.tile([P, N], BF16, tag="tp", space="PSUM")
            for nb in range(NB):
                nc.tensor.transpose(tpr[:, nb * P:(nb + 1) * P],
                                    nxT[:, nb * P:(nb + 1) * P],
                                    identity=c.identb[:])
            x1n8 = work.tile([P, NB, D], FP8, tag="x1n8")
            nc.scalar.activation(x1n8[:], tpr[:], Act.Copy, scale=64.0)
            tin8, t0T = x1n8, nxT
            yield
        else:
            nc.scalar.dma_start(dram["out_x"][g], nxT[:])


def build(bl=BL):
    nc = bacc.Bacc("TRN2", target_bir_lowering=False, debug=False,
                   enable_asserts=False, num_devices=N_CORES)
    dram = {
        "idxn": nc.dram_tensor("idxn", (bl, P, NB), I32, kind="ExternalInput").ap(),
        "idxa": nc.dram_tensor("idxa", (bl, P, KC), I32, kind="ExternalInput").ap(),
        "embed": nc.dram_tensor("embed", (NODE_VOCAB, D), BF16, kind="ExternalInput").ap(),
        "rec": nc.dram_tensor("rec", (APP_VOCAB, R), BF16, kind="ExternalInput").ap(),
        "noeyed": nc.dram_tensor("noeyed", (P, NB, N), BF16, kind="ExternalInput").ap(),
        "u1d": nc.dram_tensor("u1d", (P, 3, P), BF16, kind="ExternalInput").ap(),
        "u2d": nc.dram_tensor("u2d", (P, 3, P), BF16, kind="ExternalInput").ap(),
        "b1d": nc.dram_tensor("b1d", (P, 1), F32, kind="ExternalInput").ap(),
        "b2d": nc.dram_tensor("b2d", (P, 1), F32, kind="ExternalInput").ap(),
        "out_app": nc.dram_tensor("out_app", (bl, P, NB, R), BF16, kind="ExternalOutput").ap(),
        "out_x": nc.dram_tensor("out_x", (bl, P, N), BF16, kind="ExternalOutput").ap(),
        "dbg_mean": nc.dram_tensor("dbg_mean", (P, 1), F32, kind="ExternalOutput").ap(),
        "dbg_v5": nc.dram_tensor("dbg_v5", (P, NB), F32, kind="ExternalOutput").ap(),
        "dbg_deg": nc.dram_tensor("dbg_deg", (P, NB), F32, kind="ExternalOutput").ap(),
        "dbg_adj": nc.dram_tensor("dbg_adj", (P, NB, N), BF16, kind="ExternalOutput").ap(),
        "dbg_t1T": nc.dram_tensor("dbg_t1T", (P, N), BF16, kind="ExternalOutput").ap(),
        "dbg_yT": nc.dram_tensor("dbg_yT", (P, N), BF16, kind="ExternalOutput").ap(),
        "dbg_as": nc.dram_tensor("dbg_as", (P, N), BF16, kind="ExternalOutput").ap(),
    }
    with tile.TileContext(nc) as tc:
        with tc.tile_pool(name="const", bufs=1) as const, \
             tc.tile_pool(name="io", bufs=4) as io, \
             tc.tile_pool(name="io2", bufs=4) as io2, \
             tc.tile_pool(name="work", bufs=4) as work, \
             tc.tile_pool(name="small", bufs=8) as small, \
             tc.tile_pool(name="psG", bufs=1, space="PSUM") as psG, \
             tc.tile_pool(name="ps_tp", bufs=3, space="PSUM") as ps_tp, \
             tc.tile_pool(name="ps_mm", bufs=3, space="PSUM") as ps_mm, \
             tc.tile_pool(name="ps_sm", bufs=1, space="PSUM") as ps_sm:
            c = _consts(nc, const, dram)
            pools = (io, io2, work, small, psG, ps_tp, ps_mm, ps_sm)
            states = {}

            def drain(gens):
                # round-robin the stage generators until all exhausted
                gens = [g for g in gens if g is not None]
                while gens:
                    nxt = []
                    for gen in gens:
                        try:
                            next(gen)
                            nxt.append(gen)
                        except StopIteration:
                            pass
                    gens = nxt

            for i in range(bl + 4):
                if i < bl:
                    states[i] = {}
                    _stage_a(nc, pools, c, i, dram, states[i])
                gens = []
                if 0 <= i - 4 < bl:
                    gens.append(_stage_c(nc, pools, c, i - 4, dram, states[i - 4]))
                if 0 <= i - 3 < bl:
                    gens.append(_stage_b2b(nc, pools, c, i - 3, dram, states[i - 3]))
                if 0 <= i - 2 < bl:
                    gens.append(_stage_b2a(nc, pools, c, i - 2, dram, states[i - 2]))
                if 0 <= i - 1 < bl:
                    gens.append(_stage_b1(nc, pools, c, i - 1, dram, states[i - 1]))
                drain(gens)
                if 0 <= i - 4 < bl:
                    del states[i - 4]
    nc.compile()
    return nc


def host_inputs(input_seq, recd_token, embed_table, rec_embed_table,
                cheb_w1, cheb_b1, cheb_w2, cheb_b2, bl=BL, n_cores=N_CORES):
    seq = np.asarray(input_seq, dtype=np.int64).astype(np.int32)
    tok = np.asarray(recd_token, dtype=np.int64).astype(np.int32)
    embed = np.asarray(embed_table, np.float32).astype(ml_dtypes.bfloat16)
    rec = np.asarray(rec_embed_table, np.float32).astype(ml_dtypes.bfloat16)
    w1 = np.asarray(cheb_w1, dtype=np.float32)
    w2 = np.asarray(cheb_w2, dtype=np.float32)
    # u = [w0 - w2, w1, 2*w2]: Chebyshev t2 = 2*S*t1 - t0 fold
    u1 = np.ascontiguousarray(
        np.stack([w1[0] - w1[2], w1[1], 2.0 * w1[2]], axis=0
                 ).transpose(1, 0, 2)).astype(ml_dtypes.bfloat16)
    u2 = np.ascontiguousarray(
        np.stack([w2[0] - w2[2], w2[1], 2.0 * w2[2]], axis=0
                 ).transpose(1, 0, 2)).astype(ml_dtypes.bfloat16)
    b1 = np.asarray(cheb_b1, dtype=np.float32).reshape(P, 1)
    b2 = np.asarray(cheb_b2, dtype=np.float32).reshape(P, 1)

    pidx = np.arange(P)[:, None, None]
    nbidx = np.arange(NB)[None, :, None]
    cidx = np.arange(N)[None, None, :]
    noeyed = (cidx != nbidx * P + pidx).astype(ml_dtypes.bfloat16)

    maps = []
    for ci in range(n_cores):
        g0 = ci * bl
        idxn = np.ascontiguousarray(
            seq[g0:g0 + bl].reshape(bl, NB, P).transpose(0, 2, 1))
        idxa = np.ascontiguousarray(
            tok[g0:g0 + bl].reshape(bl, KC, P).transpose(0, 2, 1))
        maps.append({
            "idxn": idxn, "idxa": idxa, "embed": embed, "rec": rec,
            "noeyed": noeyed, "u1d": u1, "u2d": u2, "b1d": b1, "b2d": b2,
        })
    return maps


_NC_CACHE = {}


def _get_nc(bl=BL):
    if bl not in _NC_CACHE:
        _NC_CACHE[bl] = build(bl)
    return _NC_CACHE[bl]


def kernel(input_seq, recd_token, embed_table, rec_embed_table,
           cheb_w1, cheb_b1, cheb_w2, cheb_b2):
    nc = _get_nc()
    maps = host_inputs(input_seq, recd_token, embed_table, rec_embed_table,
                       cheb_w1, cheb_b1, cheb_w2, cheb_b2)
    res = run_bass_kernel_spmd(nc, maps, core_ids=list(range(N_CORES)))
    parts = []
    for ci in range(N_CORES):
        r = res.results[ci]
        ox = np.asarray(r["out_x"]).astype(np.float32)
        x = np.ascontiguousarray(ox.transpose(0, 2, 1)).reshape(BL * N, D)
        oa = np.asarray(r["out_app"]).astype(np.float32)
        oa = np.ascontiguousarray(oa.transpose(0, 2, 1, 3)).reshape(BL * N, R)
        parts.append(np.concatenate([x, oa], axis=1))
    return np.concatenate(parts, axis=0)


# revision 4
# speedup vs baseline: 1.0051x; 1.0051x over previous
"""Trainium2 Bass kernel v2 for nn_GCNDeno (per-sample cosine-graph ChebConv).

Data-parallel over B=128 graphs: 16 graphs per core. Per graph:
  - gather app rows (bf16), emit out_app (bf16, host widens)
  - normalize rows -> F (bf16, x16 scale), PE-transpose -> Ft (fp8 e4m3)
  - Gram G = Ft^T Ft via fp8 DoubleRow matmuls (G = 256*A, PSUM quarters)
  - mean of strict upper via PE matvecs (s rides the Ft-copy accums)
  - adj = (G >= mean)*noeye as FP8 (binary, exact in fp8), deg accum
  - ChebConv with dinv folded into broadcast map tiles:
      x' = 64*dinv.x (fp8); t1 = (-dinv/64).(adj@x'); t1' = -4*dinv^2 . t1;
      y' = (-dinv/64).(adj@t1') = 4*S t1; h = u0^T xT + u1^T t1T + u2^T y'T
    with u = [w0-w2, w1, w2/2] (Chebyshev t2-fold + /4 for the y' scale)
  - relu+bias on Act; layer 2 same; out_x = x2^T (bf16, host transposes)
"""
import numpy as np
import ml_dtypes

import concourse.bass as bass
import concourse.tile as tile
from concourse import bacc, mybir
from concourse.bass_utils import run_bass_kernel_spmd
from concourse.masks import make_identity

B, N, D, R = 128, 512, 128, 512
NODE_VOCAB, APP_VOCAB = 30000, 5000
N_CORES = 8
BL = B // N_CORES
EPS = 1e-12
CNT_UP = N * (N - 1) // 2
P = 128
NB = N // P
KC = R // P

F32 = mybir.dt.float32
F32R = mybir.dt.float32r
BF16 = mybir.dt.bfloat16
FP8 = mybir.dt.float8e4
I32 = mybir.dt.int32
Alu = mybir.AluOpType
Act = mybir.ActivationFunctionType
AX = mybir.AxisListType
PM = mybir.MatmulPerfMode


class _C:
    pass


def _consts(nc, const, dram):
    c = _C()
    identf = const.tile([P, P], F32)
    make_identity(nc, identf[:])
    c.identb = const.tile([P, P], BF16)
    nc.vector.tensor_copy(c.identb[:], identf[:])
    c.ones_b = const.tile([P, P], BF16)
    nc.vector.memset(c.ones_b[:], 1.0)
    c.ones_row = const.tile([1, P], BF16)
    nc.vector.memset(c.ones_row[:], 1.0)
    c.sum_sq = const.tile([P, P], F32)
    nc.vector.memset(c.sum_sq[:], 1.0 / (2.0 * CNT_UP))
    c.eps = const.tile([P, 1], F32)
    nc.vector.memset(c.eps[:], EPS)
    c.noeye = const.tile([P, NB, N], BF16)
    nc.sync.dma_start(c.noeye[:], dram["noeyed"][:])
    for w in ("u1", "u2"):
        t = const.tile([P, 3, P], BF16, tag=w)
        nc.sync.dma_start(t[:], dram[w + "d"][:])
        setattr(c, w, t)
    for bn in ("b1", "b2"):
        t = const.tile([P, 1], F32, tag=bn)
        nc.sync.dma_start(t[:], dram[bn + "d"][:])
        setattr(c, bn, t)
    return c


def _stage_a(nc, pools, c, g, dram, st):
    """Gathers + app output."""
    io, io2, work, small, psG, ps_tp, ps_mm, ps_sm = pools
    idxn = small.tile([P, NB], I32, tag="idxn")
    idxa = small.tile([P, KC], I32, tag="idxa")
    nc.sync.dma_start(idxn[:], dram["idxn"][g])
    nc.sync.dma_start(idxa[:], dram["idxa"][g])

    E = io.tile([P, NB, R], BF16, tag="E")
    for nb in range(NB):
        nc.gpsimd.indirect_dma_start(
            out=E[:, nb, :], out_offset=None, in_=dram["rec"][:],
            in_offset=bass.IndirectOffsetOnAxis(ap=idxa[:, nb:nb + 1], axis=0))
    X = io2.tile([P, NB, D], BF16, tag="X")
    for nb in range(NB):
        nc.gpsimd.indirect_dma_start(
            out=X[:, nb, :], out_offset=None, in_=dram["embed"][:],
            in_offset=bass.IndirectOffsetOnAxis(ap=idxn[:, nb:nb + 1], axis=0))
    nc.scalar.dma_start(dram["out_app"][g], E[:])
    st["E"], st["X"] = E, X


def _stage_b1(nc, pools, c, g, dram, st):
    """Normalize -> F; transpose -> Ft fp8."""
    io, io2, work, small, psG, ps_tp, ps_mm, ps_sm = pools
    E = st["E"]

    ss = small.tile([P, NB], F32, tag="ss")
    dump = small.tile([P, 2, R], BF16, tag="dump")
    for nb in range(NB):
        if nb % 2 == 0:
            nc.vector.scalar_tensor_tensor(
                out=dump[:, 0, :], in0=E[:, nb, :], scalar=1.0,
                in1=E[:, nb, :], op0=Alu.mult, op1=Alu.mult,
                accum_out=ss[:, nb:nb + 1])
        else:
            nc.scalar.activation(dump[:, 1, :], E[:, nb, :], Act.Square,
                                 accum_out=ss[:, nb:nb + 1])
    nrm16 = small.tile([P, NB], F32, tag="nrm16")
    nc.scalar.activation(nrm16[:], ss[:], Act.Sqrt, scale=1.0 / 256.0)
    inv16 = small.tile([P, NB], F32, tag="inv16")
    nc.vector.reciprocal(inv16[:], nrm16[:])
    yield

    F = io.tile([P, NB, R], BF16, tag="F")
    for nb in range(NB):
        nc.vector.tensor_scalar(F[:, nb, :], E[:, nb, :],
                                inv16[:, nb:nb + 1], 0.0,
                                op0=Alu.mult, op1=Alu.add)
    yield

    Ft = io2.tile([P, KC, N], FP8, tag="Ft")
    for kc in range(KC):
        tp = ps_tp.tile([P, N], BF16, tag="tp", space="PSUM")
        for nb in range(NB):
            nc.tensor.transpose(tp[:, nb * P:(nb + 1) * P],
                                F[:, nb, kc * P:(kc + 1) * P],
                                identity=c.identb[:])
        if kc % 2 == 0:
            nc.vector.tensor_scalar(Ft[:, kc, :], tp[:], 1.0, 0.0,
                                    op0=Alu.mult, op1=Alu.add)
        else:
            nc.scalar.copy(Ft[:, kc, :], tp[:])
        if kc == 1:
            yield
    st["Ft"] = Ft


def _stage_b2a(nc, pools, c, g, dram, st):
    """Gram quarters -> As (bf16 SBUF) with rowsum accums -> mean."""
    io, io2, work, small, psG, ps_tp, ps_mm, ps_sm = pools
    Ft = st["Ft"]

    # Gram quarters -> As bf16 (SBUF), noeye folded (zero diagonal);
    # v5 off-diagonal rowsums ride the copies
    As = io2.tile([P, NB, N], BF16, tag="As")
    v5 = small.tile([P, NB], F32, tag="v5")
    gc_eng = [nc.vector, nc.vector, nc.vector, nc.vector]
    for nb in range(NB):
        gq = psG.tile([P, N], F32, tag="G", space="PSUM")
        for k in range(2):
            nc.tensor.matmul(gq[:],
                             lhsT=Ft[:, 2 * k:2 * k + 2, nb * P:(nb + 1) * P],
                             rhs=Ft[:, 2 * k:2 * k + 2, :],
                             start=(k == 0), stop=(k == 1),
                             perf_mode=PM.DoubleRow)
        gc_eng[nb].scalar_tensor_tensor(
            out=As[:, nb, :], in0=gq[:], scalar=1.0,
            in1=c.noeye[:, nb, :], op0=Alu.mult, op1=Alu.mult,
            accum_out=v5[:, nb:nb + 1])
        if nb == 1:
            yield
    mps = ps_sm.tile([P, 8], F32, tag="sm", space="PSUM")
    nc.tensor.matmul(mps[:, 0:NB], lhsT=c.sum_sq[:], rhs=v5[:],
                     start=True, stop=True)
    mean_bc = small.tile([P, 1], F32, tag="mean")
    nc.vector.tensor_reduce(mean_bc[:], mps[:, 0:NB], axis=AX.X, op=Alu.add)
    if g == 0:
        nc.sync.dma_start(dram["dbg_mean"][:], mean_bc[:])
        nc.sync.dma_start(dram["dbg_v5"][:], v5[:])
        nc.sync.dma_start(dram["dbg_as"][:], As[:, 0, :])
    st.update(As=As, mean=mean_bc)


def _stage_b2b(nc, pools, c, g, dram, st):
    """Adjacency (bf16), dinv, S8 (fp8, x256), x8, xT."""
    io, io2, work, small, psG, ps_tp, ps_mm, ps_sm = pools
    X, As, mean_bc = st["X"], st["As"], st["mean"]

    adj = io2.tile([P, NB, N], BF16, tag="adj")
    deg = small.tile([P, NB], F32, tag="deg")
    for nb in range(NB):
        nc.vector.scalar_tensor_tensor(
            out=adj[:, nb, :], in0=As[:, nb, :], scalar=mean_bc[:, :1],
            in1=c.noeye[:, nb, :], op0=Alu.is_ge, op1=Alu.mult,
            accum_out=deg[:, nb:nb + 1])
    yield

    # dinv chain; dinv16 = 16/sqrt(deg)
    nrmd = small.tile([P, NB], F32, tag="nrmd")
    nc.scalar.activation(nrmd[:], deg[:], Act.Sqrt, bias=c.eps[:, :1],
                         scale=1.0 / 256.0)
    dinv16 = small.tile([P, NB], F32, tag="dinv16")
    nc.vector.reciprocal(dinv16[:], nrmd[:])
    # ndbc16[?, m] = 16*dinv_m broadcast across partitions
    dT = ps_tp.tile([P, N], BF16, tag="tp", space="PSUM")
    db = small.tile([P, NB], BF16, tag="dinvb")
    nc.vector.tensor_copy(db[:], dinv16[:])
    for nb in range(NB):
        nc.tensor.transpose(dT[0:1, nb * P:(nb + 1) * P], db[:, nb:nb + 1],
                            identity=c.identb[:])
    dTb = small.tile([1, N], BF16, tag="dTb")
    nc.vector.tensor_copy(dTb[:], dT[0:1, 0:N])
    nd_ps = psG.tile([P, N], F32, tag="G", space="PSUM")
    nc.tensor.matmul(nd_ps[:], lhsT=c.ones_row[:], rhs=dTb[:],
                     start=True, stop=True)
    ndbc = small.tile([P, N], BF16, tag="ndbc")
    nc.scalar.copy(ndbc[:], nd_ps[:])
    yield

    # S8 = -(16 dinv_n) * adj * (16 dinv_m) / 256 scale-carried (fp8)
    nds = small.tile([P, NB], F32, tag="nds")
    nc.vector.tensor_scalar(nds[:], dinv16[:], -1.0, 0.0, op0=Alu.mult, op1=Alu.add)
    S8 = io2.tile([P, NB, N], FP8, tag="S8")
    for nb in range(NB):
        nc.vector.scalar_tensor_tensor(
            out=S8[:, nb, :], in0=adj[:, nb, :], scalar=nds[:, nb:nb + 1],
            in1=ndbc[:], op0=Alu.mult, op1=Alu.mult)
    yield

    # x8 = 64*x (fp8), xT (bf16)
    x8 = work.tile([P, NB, D], FP8, tag="x8")
    nc.scalar.activation(x8[:], X[:], Act.Copy, scale=64.0)
    tpx = ps_tp.tile([P, N], BF16, tag="tp", space="PSUM")
    for nb in range(NB):
        nc.tensor.transpose(tpx[:, nb * P:(nb + 1) * P], X[:, nb, :],
                            identity=c.identb[:])
    xT = work.tile([P, N], BF16, tag="xT")
    nc.scalar.copy(xT[:], tpx[:])
    if g == 0:
        nc.sync.dma_start(dram["dbg_deg"][:], deg[:])
        nc.sync.dma_start(dram["dbg_adj"][:], adj[:])
    st.update(x8=x8, xT=xT, S8=S8)


def _stage_c(nc, pools, c, g, dram, st):
    """Two ChebConv layers, transposed-out applies vs S8."""
    io, io2, work, small, psG, ps_tp, ps_mm, ps_sm = pools
    S8 = st["S8"]
    US = 1.0 / (256.0 * 64.0)    # un-scale for S8(x256) @ (x64) operands

    tin8, t0T = st["x8"], st["xT"]
    for layer, (u, b) in enumerate(((c.u1, c.b1), (c.u2, c.b2))):
        last = layer == 1
        # t1^T = (S t0)^T : [d, n] psum
        p1 = ps_mm.tile([P, N], F32, tag="mm", space="PSUM")
        for k in range(2):
            nc.tensor.matmul(p1[:], lhsT=tin8[:, 2 * k:2 * k + 2, :],
                             rhs=S8[:, 2 * k:2 * k + 2, :],
                             start=(k == 0), stop=(k == 1),
                             perf_mode=PM.DoubleRow)
        yield
        t1T = work.tile([P, N], BF16, tag=f"t1T_{layer}")
        nc.scalar.activation(t1T[:], p1[:], Act.Copy, scale=US)
        if g == 0 and layer == 0:
            nc.sync.dma_start(dram["dbg_t1T"][:], t1T[:])
        tpn = ps_tp.tile([P, N], BF16, tag="tp", space="PSUM")
        for nb in range(NB):
            nc.tensor.transpose(tpn[:, nb * P:(nb + 1) * P],
                                t1T[:, nb * P:(nb + 1) * P],
                                identity=c.identb[:])
        t1n8 = work.tile([P, NB, D], FP8, tag=f"t1n8_{layer}")
        nc.scalar.activation(t1n8[:], tpn[:], Act.Copy, scale=64.0)
        yield
        # y^T = (S t1)^T
        p2 = ps_mm.tile([P, N], F32, tag="mm", space="PSUM")
        for k in range(2):
            nc.tensor.matmul(p2[:], lhsT=t1n8[:, 2 * k:2 * k + 2, :],
                             rhs=S8[:, 2 * k:2 * k + 2, :],
                             start=(k == 0), stop=(k == 1),
                             perf_mode=PM.DoubleRow)
        yT = work.tile([P, N], BF16, tag=f"yT_{layer}")
        if layer == 0:
            nc.scalar.activation(yT[:], p2[:], Act.Copy, scale=US)
            if g == 0:
                nc.sync.dma_start(dram["dbg_yT"][:], yT[:])
        else:
            nc.vector.tensor_scalar(yT[:], p2[:], US, 0.0,
                                    op0=Alu.mult, op1=Alu.add)
        yield
        # h^T = u0^T t0T + u1^T t1T + u2^T yT (+b, relu)
        hp = ps_mm.tile([P, N], F32, tag="mm", space="PSUM")
        nc.tensor.matmul(hp[:], lhsT=u[:, 0, :], rhs=t0T[:], start=True, stop=False)
        nc.tensor.matmul(hp[:], lhsT=u[:, 1, :], rhs=t1T[:], start=False, stop=False)
        nc.tensor.matmul(hp[:], lhsT=u[:, 2, :], rhs=yT[:], start=False, stop=True)
        nxT = work.tile([P, N], BF16, tag=f"nxT_{layer}")
        nc.scalar.activation(nxT[:], hp[:], Act.Relu, bias=b[:, :1])
        if not last:
            tpr = ps_tp.tile([P, N], BF16, tag="tp", space="PSUM")
            for nb in range(NB):
                nc.tensor.transpose(tpr[:, nb * P:(nb + 1) * P],
                                    nxT[:, nb * P:(nb + 1) * P],
                                    identity=c.identb[:])
            x1n8 = work.tile([P, NB, D], FP8, tag="x1n8")
            nc.scalar.activation(x1n8[:], tpr[:], Act.Copy, scale=64.0)
            tin8, t0T = x1n8, nxT
            yield
        else:
            nc.scalar.dma_start(dram["out_x"][g], nxT[:])


def build(bl=BL):
    nc = bacc.Bacc("TRN2", target_bir_lowering=False, debug=False,
                   enable_asserts=False, num_devices=N_CORES)
    dram = {
        "idxn": nc.dram_tensor("idxn", (bl, P, NB), I32, kind="ExternalInput").ap(),
        "idxa": nc.dram_tensor("idxa", (bl, P, KC), I32, kind="ExternalInput").ap(),
        "embed": nc.dram_tensor("embed", (NODE_VOCAB, D), BF16, kind="ExternalInput").ap(),
        "rec": nc.dram_tensor("rec", (APP_VOCAB, R), BF16, kind="ExternalInput").ap(),
        "noeyed": nc.dram_tensor("noeyed", (P, NB, N), BF16, kind="ExternalInput").ap(),
        "u1d": nc.dram_tensor("u1d", (P, 3, P), BF16, kind="ExternalInput").ap(),
        "u2d": nc.dram_tensor("u2d", (P, 3, P), BF16, kind="ExternalInput").ap(),
        "b1d": nc.dram_tensor("b1d", (P, 1), F32, kind="ExternalInput").ap(),
        "b2d": nc.dram_tensor("b2d", (P, 1), F32, kind="ExternalInput").ap(),
        "out_app": nc.dram_tensor("out_app", (bl, P, NB, R), BF16, kind="ExternalOutput").ap(),
        "out_x": nc.dram_tensor("out_x", (bl, P, N), BF16, kind="ExternalOutput").ap(),
        "dbg_mean": nc.dram_tensor("dbg_mean", (P, 1), F32, kind="ExternalOutput").ap(),
        "dbg_v5": nc.dram_tensor("dbg_v5", (P, NB), F32, kind="ExternalOutput").ap(),
        "dbg_deg": nc.dram_tensor("dbg_deg", (P, NB), F32, kind="ExternalOutput").ap(),
        "dbg_adj": nc.dram_tensor("dbg_adj", (P, NB, N), BF16, kind="ExternalOutput").ap(),
        "dbg_t1T": nc.dram_tensor("dbg_t1T", (P, N), BF16, kind="ExternalOutput").ap(),
        "dbg_yT": nc.dram_tensor("dbg_yT", (P, N), BF16, kind="ExternalOutput").ap(),
        "dbg_as": nc.dram_tensor("dbg_as", (P, N), BF16, kind="ExternalOutput").ap(),
    }
    with tile.TileContext(nc) as tc:
        with tc.tile_pool(name="const", bufs=1) as const, \
             tc.tile_pool(name="io", bufs=4) as io, \
             tc.tile_pool(name="io2", bufs=4) as io2, \
             tc.tile_pool(name="work", bufs=4) as work, \
             tc.tile_pool(name="small", bufs=12) as small, \
             tc.tile_pool(name="psG", bufs=1, space="PSUM") as psG, \
             tc.tile_pool(name="ps_tp", bufs=3, space="PSUM") as ps_tp, \
             tc.tile_pool(name="ps_mm", bufs=3, space="PSUM") as ps_mm, \
             tc.tile_pool(name="ps_sm", bufs=1, space="PSUM") as ps_sm:
            c = _consts(nc, const, dram)
            pools = (io, io2, work, small, psG, ps_tp, ps_mm, ps_sm)
            states = {}

            def drain(gens):
                # round-robin the stage generators until all exhausted
                gens = [g for g in gens if g is not None]
                while gens:
                    nxt = []
                    for gen in gens:
                        try:
                            next(gen)
                            nxt.append(gen)
                        except StopIteration:
                            pass
                    gens = nxt

            for i in range(bl + 4):
                if i < bl:
                    states[i] = {}
                    _stage_a(nc, pools, c, i, dram, states[i])
                gens = []
                if 0 <= i - 4 < bl:
                    gens.append(_stage_c(nc, pools, c, i - 4, dram, states[i - 4]))
                if 0 <= i - 3 < bl:
                    gens.append(_stage_b2b(nc, pools, c, i - 3, dram, states[i - 3]))
                if 0 <= i - 2 < bl:
                    gens.append(_stage_b2a(nc, pools, c, i - 2, dram, states[i - 2]))
                if 0 <= i - 1 < bl:
                    gens.append(_stage_b1(nc, pools, c, i - 1, dram, states[i - 1]))
                drain(gens)
                if 0 <= i - 4 < bl:
                    del states[i - 4]
    nc.compile()
    return nc


def host_inputs(input_seq, recd_token, embed_table, rec_embed_table,
                cheb_w1, cheb_b1, cheb_w2, cheb_b2, bl=BL, n_cores=N_CORES):
    seq = np.asarray(input_seq, dtype=np.int64).astype(np.int32)
    tok = np.asarray(recd_token, dtype=np.int64).astype(np.int32)
    embed = np.asarray(embed_table, np.float32).astype(ml_dtypes.bfloat16)
    rec = np.asarray(rec_embed_table, np.float32).astype(ml_dtypes.bfloat16)
    w1 = np.asarray(cheb_w1, dtype=np.float32)
    w2 = np.asarray(cheb_w2, dtype=np.float32)
    # u = [w0 - w2, w1, 2*w2]: Chebyshev t2 = 2*S*t1 - t0 fold
    u1 = np.ascontiguousarray(
        np.stack([w1[0] - w1[2], w1[1], 2.0 * w1[2]], axis=0
                 ).transpose(1, 0, 2)).astype(ml_dtypes.bfloat16)
    u2 = np.ascontiguousarray(
        np.stack([w2[0] - w2[2], w2[1], 2.0 * w2[2]], axis=0
                 ).transpose(1, 0, 2)).astype(ml_dtypes.bfloat16)
    b1 = np.asarray(cheb_b1, dtype=np.float32).reshape(P, 1)
    b2 = np.asarray(cheb_b2, dtype=np.float32).reshape(P, 1)

    pidx = np.arange(P)[:, None, None]
    nbidx = np.arange(NB)[None, :, None]
    cidx = np.arange(N)[None, None, :]
    noeyed = (cidx != nbidx * P + pidx).astype(ml_dtypes.bfloat16)

    maps = []
    for ci in range(n_cores):
        g0 = ci * bl
        idxn = np.ascontiguousarray(
            seq[g0:g0 + bl].reshape(bl, NB, P).transpose(0, 2, 1))
        idxa = np.ascontiguousarray(
            tok[g0:g0 + bl].reshape(bl, KC, P).transpose(0, 2, 1))
        maps.append({
            "idxn": idxn, "idxa": idxa, "embed": embed, "rec": rec,
            "noeyed": noeyed, "u1d": u1, "u2d": u2, "b1d": b1, "b2d": b2,
        })
    return maps


_NC_CACHE = {}


def _get_nc(bl=BL):
    if bl not in _NC_CACHE:
        _NC_CACHE[bl] = build(bl)
    return _NC_CACHE[bl]


def kernel(input_seq, recd_token, embed_table, rec_embed_table,
           cheb_w1, cheb_b1, cheb_w2, cheb_b2):
    nc = _get_nc()
    maps = host_inputs(input_seq, recd_token, embed_table, rec_embed_table,
                       cheb_w1, cheb_b1, cheb_w2, cheb_b2)
    res = run_bass_kernel_spmd(nc, maps, core_ids=list(range(N_CORES)))
    parts = []
    for ci in range(N_CORES):
        r = res.results[ci]
        ox = np.asarray(r["out_x"]).astype(np.float32)
        x = np.ascontiguousarray(ox.transpose(0, 2, 1)).reshape(BL * N, D)
        oa = np.asarray(r["out_app"]).astype(np.float32)
        oa = np.ascontiguousarray(oa.transpose(0, 2, 1, 3)).reshape(BL * N, R)
        parts.append(np.concatenate([x, oa], axis=1))
    return np.concatenate(parts, axis=0)


# revision 5
# speedup vs baseline: 1.0112x; 1.0061x over previous
"""Trainium2 Bass kernel v2 for nn_GCNDeno (per-sample cosine-graph ChebConv).

Data-parallel over B=128 graphs: 16 graphs per core. Per graph:
  - gather app rows (bf16), emit out_app (bf16, host widens)
  - normalize rows -> F (bf16, x16 scale), PE-transpose -> Ft (fp8 e4m3)
  - Gram G = Ft^T Ft via fp8 DoubleRow matmuls (G = 256*A, PSUM quarters)
  - mean of strict upper via PE matvecs (s rides the Ft-copy accums)
  - adj = (G >= mean)*noeye as FP8 (binary, exact in fp8), deg accum
  - ChebConv with dinv folded into broadcast map tiles:
      x' = 64*dinv.x (fp8); t1 = (-dinv/64).(adj@x'); t1' = -4*dinv^2 . t1;
      y' = (-dinv/64).(adj@t1') = 4*S t1; h = u0^T xT + u1^T t1T + u2^T y'T
    with u = [w0-w2, w1, w2/2] (Chebyshev t2-fold + /4 for the y' scale)
  - relu+bias on Act; layer 2 same; out_x = x2^T (bf16, host transposes)
"""
import numpy as np
import ml_dtypes

import concourse.bass as bass
import concourse.tile as tile
from concourse import bacc, mybir
from concourse.bass_utils import run_bass_kernel_spmd
from concourse.masks import make_identity

B, N, D, R = 128, 512, 128, 512
NODE_VOCAB, APP_VOCAB = 30000, 5000
N_CORES = 8
BL = B // N_CORES
EPS = 1e-12
CNT_UP = N * (N - 1) // 2
P = 128
NB = N // P
KC = R // P

F32 = mybir.dt.float32
F32R = mybir.dt.float32r
BF16 = mybir.dt.bfloat16
FP8 = mybir.dt.float8e4
I32 = mybir.dt.int32
Alu = mybir.AluOpType
Act = mybir.ActivationFunctionType
AX = mybir.AxisListType
PM = mybir.MatmulPerfMode


class _C:
    pass


def _consts(nc, const, dram):
    c = _C()
    identf = const.tile([P, P], F32)
    make_identity(nc, identf[:])
    c.identb = const.tile([P, P], BF16)
    nc.vector.tensor_copy(c.identb[:], identf[:])
    c.ones_b = const.tile([P, P], BF16)
    nc.vector.memset(c.ones_b[:], 1.0)
    c.ones_row = const.tile([1, P], BF16)
    nc.vector.memset(c.ones_row[:], 1.0)
    c.sum_sq = const.tile([P, P], F32)
    nc.vector.memset(c.sum_sq[:], 1.0 / (2.0 * CNT_UP))
    c.eps = const.tile([P, 1], F32)
    nc.vector.memset(c.eps[:], EPS)
    c.noeye = const.tile([P, NB, N], BF16)
    nc.sync.dma_start(c.noeye[:], dram["noeyed"][:])
    for w in ("u1", "u2"):
        t = const.tile([P, 3, P], BF16, tag=w)
        nc.sync.dma_start(t[:], dram[w + "d"][:])
        setattr(c, w, t)
    for bn in ("b1", "b2"):
        t = const.tile([P, 1], F32, tag=bn)
        nc.sync.dma_start(t[:], dram[bn + "d"][:])
        setattr(c, bn, t)
    return c


def _stage_a(nc, pools, c, g, dram, st):
    """Gathers + app output."""
    io, io2, work, small, psG, ps_tp, ps_mm, ps_sm = pools
    idxn = small.tile([P, NB], I32, tag="idxn")
    idxa = small.tile([P, KC], I32, tag="idxa")
    nc.sync.dma_start(idxn[:], dram["idxn"][g])
    nc.sync.dma_start(idxa[:], dram["idxa"][g])

    E = io.tile([P, NB, R], BF16, tag="E")
    for nb in range(NB):
        nc.gpsimd.indirect_dma_start(
            out=E[:, nb, :], out_offset=None, in_=dram["rec"][:],
            in_offset=bass.IndirectOffsetOnAxis(ap=idxa[:, nb:nb + 1], axis=0))
    X = io2.tile([P, NB, D], BF16, tag="X")
    for nb in range(NB):
        nc.gpsimd.indirect_dma_start(
            out=X[:, nb, :], out_offset=None, in_=dram["embed"][:],
            in_offset=bass.IndirectOffsetOnAxis(ap=idxn[:, nb:nb + 1], axis=0))
    nc.scalar.dma_start(dram["out_app"][g], E[:])
    st["E"], st["X"] = E, X


def _stage_b1(nc, pools, c, g, dram, st):
    """Normalize -> F; transpose -> Ft fp8."""
    io, io2, work, small, psG, ps_tp, ps_mm, ps_sm = pools
    E = st["E"]

    ss = small.tile([P, NB], F32, tag="ss")
    dump = small.tile([P, 2, R], BF16, tag="dump")
    for nb in range(NB):
        if nb % 2 == 0:
            nc.vector.scalar_tensor_tensor(
                out=dump[:, 0, :], in0=E[:, nb, :], scalar=1.0,
                in1=E[:, nb, :], op0=Alu.mult, op1=Alu.mult,
                accum_out=ss[:, nb:nb + 1])
        else:
            nc.scalar.activation(dump[:, 1, :], E[:, nb, :], Act.Square,
                                 accum_out=ss[:, nb:nb + 1])
    nrm16 = small.tile([P, NB], F32, tag="nrm16")
    nc.scalar.activation(nrm16[:], ss[:], Act.Sqrt, scale=1.0 / 256.0)
    inv16 = small.tile([P, NB], F32, tag="inv16")
    nc.vector.reciprocal(inv16[:], nrm16[:])
    yield

    F = io.tile([P, NB, R], BF16, tag="F")
    for nb in range(NB):
        nc.vector.tensor_scalar(F[:, nb, :], E[:, nb, :],
                                inv16[:, nb:nb + 1], 0.0,
                                op0=Alu.mult, op1=Alu.add)
    yield

    Ft = io2.tile([P, KC, N], FP8, tag="Ft")
    for kc in range(KC):
        tp = ps_tp.tile([P, N], BF16, tag="tp", space="PSUM")
        for nb in range(NB):
            nc.tensor.transpose(tp[:, nb * P:(nb + 1) * P],
                                F[:, nb, kc * P:(kc + 1) * P],
                                identity=c.identb[:])
        if kc % 2 == 0:
            nc.vector.tensor_scalar(Ft[:, kc, :], tp[:], 1.0, 0.0,
                                    op0=Alu.mult, op1=Alu.add)
        else:
            nc.scalar.copy(Ft[:, kc, :], tp[:])
        if kc == 1:
            yield
    st["Ft"] = Ft


def _stage_b2a(nc, pools, c, g, dram, st):
    """Gram quarters -> As (bf16 SBUF) with rowsum accums -> mean."""
    io, io2, work, small, psG, ps_tp, ps_mm, ps_sm = pools
    Ft = st["Ft"]

    # Gram quarters -> As bf16 (SBUF), noeye folded (zero diagonal);
    # v5 off-diagonal rowsums ride the copies
    As = io2.tile([P, NB, N], BF16, tag="As")
    v5 = small.tile([P, NB], F32, tag="v5")
    gc_eng = [nc.vector, nc.vector, nc.vector, nc.vector]
    for nb in range(NB):
        gq = psG.tile([P, N], F32, tag="G", space="PSUM")
        for k in range(2):
            nc.tensor.matmul(gq[:],
                             lhsT=Ft[:, 2 * k:2 * k + 2, nb * P:(nb + 1) * P],
                             rhs=Ft[:, 2 * k:2 * k + 2, :],
                             start=(k == 0), stop=(k == 1),
                             perf_mode=PM.DoubleRow)
        gc_eng[nb].scalar_tensor_tensor(
            out=As[:, nb, :], in0=gq[:], scalar=1.0,
            in1=c.noeye[:, nb, :], op0=Alu.mult, op1=Alu.mult,
            accum_out=v5[:, nb:nb + 1])
        if nb == 1:
            yield
    mps = ps_sm.tile([P, 8], F32, tag="sm", space="PSUM")
    nc.tensor.matmul(mps[:, 0:NB], lhsT=c.sum_sq[:], rhs=v5[:],
                     start=True, stop=True)
    mean_bc = small.tile([P, 1], F32, tag="mean")
    nc.vector.tensor_reduce(mean_bc[:], mps[:, 0:NB], axis=AX.X, op=Alu.add)
    if g == 0:
        nc.sync.dma_start(dram["dbg_mean"][:], mean_bc[:])
        nc.sync.dma_start(dram["dbg_v5"][:], v5[:])
        nc.sync.dma_start(dram["dbg_as"][:], As[:, 0, :])
    st.update(As=As, mean=mean_bc)


def _stage_b2b(nc, pools, c, g, dram, st):
    """Adjacency (bf16), dinv, S8 (fp8, x256), x8, xT."""
    io, io2, work, small, psG, ps_tp, ps_mm, ps_sm = pools
    X, As, mean_bc = st["X"], st["As"], st["mean"]

    adj = io2.tile([P, NB, N], BF16, tag="adj")
    deg = small.tile([P, NB], F32, tag="deg")
    for nb in range(NB):
        nc.vector.scalar_tensor_tensor(
            out=adj[:, nb, :], in0=As[:, nb, :], scalar=mean_bc[:, :1],
            in1=c.noeye[:, nb, :], op0=Alu.is_ge, op1=Alu.mult,
            accum_out=deg[:, nb:nb + 1])
    yield

    # dinv chain; dinv16 = 16/sqrt(deg)
    nrmd = small.tile([P, NB], F32, tag="nrmd")
    nc.scalar.activation(nrmd[:], deg[:], Act.Sqrt, bias=c.eps[:, :1],
                         scale=1.0 / 256.0)
    dinv16 = small.tile([P, NB], F32, tag="dinv16")
    nc.vector.reciprocal(dinv16[:], nrmd[:])
    # ndbc16[?, m] = 16*dinv_m broadcast across partitions
    dT = ps_tp.tile([P, N], BF16, tag="tp", space="PSUM")
    db = small.tile([P, NB], BF16, tag="dinvb")
    nc.vector.tensor_copy(db[:], dinv16[:])
    for nb in range(NB):
        nc.tensor.transpose(dT[0:1, nb * P:(nb + 1) * P], db[:, nb:nb + 1],
                            identity=c.identb[:])
    dTb = small.tile([1, N], BF16, tag="dTb")
    nc.vector.tensor_copy(dTb[:], dT[0:1, 0:N])
    nd_ps = psG.tile([P, N], F32, tag="G", space="PSUM")
    nc.tensor.matmul(nd_ps[:], lhsT=c.ones_row[:], rhs=dTb[:],
                     start=True, stop=True)
    ndbc = small.tile([P, N], BF16, tag="ndbc")
    nc.scalar.copy(ndbc[:], nd_ps[:])
    yield

    # S8 = -(16 dinv_n) * adj * (16 dinv_m) / 256 scale-carried (fp8)
    nds = small.tile([P, NB], F32, tag="nds")
    nc.vector.tensor_scalar(nds[:], dinv16[:], -1.0, 0.0, op0=Alu.mult, op1=Alu.add)
    S8 = io2.tile([P, NB, N], FP8, tag="S8")
    for nb in range(NB):
        nc.vector.scalar_tensor_tensor(
            out=S8[:, nb, :], in0=adj[:, nb, :], scalar=nds[:, nb:nb + 1],
            in1=ndbc[:], op0=Alu.mult, op1=Alu.mult)
    yield

    # x8 = 64*x (fp8), xT (bf16)
    x8 = work.tile([P, NB, D], FP8, tag="x8")
    nc.scalar.activation(x8[:], X[:], Act.Copy, scale=64.0)
    tpx = ps_tp.tile([P, N], BF16, tag="tp", space="PSUM")
    for nb in range(NB):
        nc.tensor.transpose(tpx[:, nb * P:(nb + 1) * P], X[:, nb, :],
                            identity=c.identb[:])
    xT = work.tile([P, N], BF16, tag="xT")
    nc.scalar.copy(xT[:], tpx[:])
    if g == 0:
        nc.sync.dma_start(dram["dbg_deg"][:], deg[:])
        nc.sync.dma_start(dram["dbg_adj"][:], adj[:])
    st.update(x8=x8, xT=xT, S8=S8)


def _stage_c(nc, pools, c, g, dram, st):
    """Two ChebConv layers, transposed-out applies vs S8."""
    io, io2, work, small, psG, ps_tp, ps_mm, ps_sm = pools
    S8 = st["S8"]
    US = 1.0 / (256.0 * 64.0)    # un-scale for S8(x256) @ (x64) operands

    tin8, t0T = st["x8"], st["xT"]
    for layer, (u, b) in enumerate(((c.u1, c.b1), (c.u2, c.b2))):
        last = layer == 1
        # t1^T = (S t0)^T : [d, n] psum
        p1 = ps_mm.tile([P, N], F32, tag="mm", space="PSUM")
        for k in range(2):
            nc.tensor.matmul(p1[:], lhsT=tin8[:, 2 * k:2 * k + 2, :],
                             rhs=S8[:, 2 * k:2 * k + 2, :],
                             start=(k == 0), stop=(k == 1),
                             perf_mode=PM.DoubleRow)
        yield
        t1T = work.tile([P, N], BF16, tag=f"t1T_{layer}")
        nc.scalar.activation(t1T[:], p1[:], Act.Copy, scale=US)
        if g == 0 and layer == 0:
            nc.sync.dma_start(dram["dbg_t1T"][:], t1T[:])
        tpn = ps_tp.tile([P, N], BF16, tag="tp", space="PSUM")
        for nb in range(NB):
            nc.tensor.transpose(tpn[:, nb * P:(nb + 1) * P],
                                t1T[:, nb * P:(nb + 1) * P],
                                identity=c.identb[:])
        t1n8 = work.tile([P, NB, D], FP8, tag=f"t1n8_{layer}")
        nc.scalar.activation(t1n8[:], tpn[:], Act.Copy, scale=64.0)
        yield
        # y^T = (S t1)^T
        p2 = ps_mm.tile([P, N], F32, tag="mm", space="PSUM")
        for k in range(2):
            nc.tensor.matmul(p2[:], lhsT=t1n8[:, 2 * k:2 * k + 2, :],
                             rhs=S8[:, 2 * k:2 * k + 2, :],
                             start=(k == 0), stop=(k == 1),
                             perf_mode=PM.DoubleRow)
        yT = work.tile([P, N], BF16, tag=f"yT_{layer}")
        if layer == 0:
            nc.scalar.activation(yT[:], p2[:], Act.Copy, scale=US)
            if g == 0:
                nc.sync.dma_start(dram["dbg_yT"][:], yT[:])
        else:
            nc.vector.tensor_scalar(yT[:], p2[:], US, 0.0,
                                    op0=Alu.mult, op1=Alu.add)
        yield
        # h^T = u0^T t0T + u1^T t1T + u2^T yT (+b, relu)
        hp = ps_mm.tile([P, N], F32, tag="mm", space="PSUM")
        nc.tensor.matmul(hp[:], lhsT=u[:, 0, :], rhs=t0T[:], start=True, stop=False)
        nc.tensor.matmul(hp[:], lhsT=u[:, 1, :], rhs=t1T[:], start=False, stop=False)
        nc.tensor.matmul(hp[:], lhsT=u[:, 2, :], rhs=yT[:], start=False, stop=True)
        nxT = work.tile([P, N], BF16, tag=f"nxT_{layer}")
        nc.scalar.activation(nxT[:], hp[:], Act.Relu, bias=b[:, :1])
        if not last:
            tpr = ps_tp.tile([P, N], BF16, tag="tp", space="PSUM")
            for nb in range(NB):
                nc.tensor.transpose(tpr[:, nb * P:(nb + 1) * P],
                                    nxT[:, nb * P:(nb + 1) * P],
                                    identity=c.identb[:])
            x1n8 = work.tile([P, NB, D], FP8, tag="x1n8")
            nc.scalar.activation(x1n8[:], tpr[:], Act.Copy, scale=64.0)
            tin8, t0T = x1n8, nxT
            yield
        else:
            nc.scalar.dma_start(dram["out_x"][g], nxT[:])


def build(bl=BL):
    nc = bacc.Bacc("TRN2", target_bir_lowering=False, debug=False,
                   enable_asserts=False, num_devices=N_CORES)
    dram = {
        "idxn": nc.dram_tensor("idxn", (bl, P, NB), I32, kind="ExternalInput").ap(),
        "idxa": nc.dram_tensor("idxa", (bl, P, KC), I32, kind="ExternalInput").ap(),
        "embed": nc.dram_tensor("embed", (NODE_VOCAB, D), BF16, kind="ExternalInput").ap(),
        "rec": nc.dram_tensor("rec", (APP_VOCAB, R), BF16, kind="ExternalInput").ap(),
        "noeyed": nc.dram_tensor("noeyed", (P, NB, N), BF16, kind="ExternalInput").ap(),
        "u1d": nc.dram_tensor("u1d", (P, 3, P), BF16, kind="ExternalInput").ap(),
        "u2d": nc.dram_tensor("u2d", (P, 3, P), BF16, kind="ExternalInput").ap(),
        "b1d": nc.dram_tensor("b1d", (P, 1), F32, kind="ExternalInput").ap(),
        "b2d": nc.dram_tensor("b2d", (P, 1), F32, kind="ExternalInput").ap(),
        "out_app": nc.dram_tensor("out_app", (bl, P, NB, R), BF16, kind="ExternalOutput").ap(),
        "out_x": nc.dram_tensor("out_x", (bl, P, N), BF16, kind="ExternalOutput").ap(),
        "dbg_mean": nc.dram_tensor("dbg_mean", (P, 1), F32, kind="ExternalOutput").ap(),
        "dbg_v5": nc.dram_tensor("dbg_v5", (P, NB), F32, kind="ExternalOutput").ap(),
        "dbg_deg": nc.dram_tensor("dbg_deg", (P, NB), F32, kind="ExternalOutput").ap(),
        "dbg_adj": nc.dram_tensor("dbg_adj", (P, NB, N), BF16, kind="ExternalOutput").ap(),
        "dbg_t1T": nc.dram_tensor("dbg_t1T", (P, N), BF16, kind="ExternalOutput").ap(),
        "dbg_yT": nc.dram_tensor("dbg_yT", (P, N), BF16, kind="ExternalOutput").ap(),
        "dbg_as": nc.dram_tensor("dbg_as", (P, N), BF16, kind="ExternalOutput").ap(),
    }
    with tile.TileContext(nc) as tc:
        with tc.tile_pool(name="const", bufs=1) as const, \
             tc.tile_pool(name="io", bufs=4) as io, \
             tc.tile_pool(name="io2", bufs=4) as io2, \
             tc.tile_pool(name="work", bufs=4) as work, \
             tc.tile_pool(name="small", bufs=12) as small, \
             tc.tile_pool(name="psG", bufs=1, space="PSUM") as psG, \
             tc.tile_pool(name="ps_tp", bufs=3, space="PSUM") as ps_tp, \
             tc.tile_pool(name="ps_mm", bufs=3, space="PSUM") as ps_mm, \
             tc.tile_pool(name="ps_sm", bufs=1, space="PSUM") as ps_sm:
            c = _consts(nc, const, dram)
            pools = (io, io2, work, small, psG, ps_tp, ps_mm, ps_sm)
            states = {}

            def drain(gens):
                # round-robin the stage generators until all exhausted
                gens = [g for g in gens if g is not None]
                while gens:
                    nxt = []
                    for gen in gens:
                        try:
                            next(gen)
                            nxt.append(gen)
                        except StopIteration:
                            pass
                    gens = nxt

            for i in range(bl + 4):
                if i < bl:
                    states[i] = {}
                    _stage_a(nc, pools, c, i, dram, states[i])
                gens = []
                if 0 <= i - 1 < bl:
                    gens.append(_stage_b1(nc, pools, c, i - 1, dram, states[i - 1]))
                if 0 <= i - 4 < bl:
                    gens.append(_stage_c(nc, pools, c, i - 4, dram, states[i - 4]))
                if 0 <= i - 2 < bl:
                    gens.append(_stage_b2a(nc, pools, c, i - 2, dram, states[i - 2]))
                if 0 <= i - 3 < bl:
                    gens.append(_stage_b2b(nc, pools, c, i - 3, dram, states[i - 3]))
                drain(gens)
                if 0 <= i - 4 < bl:
                    del states[i - 4]
    nc.compile()
    return nc


def host_inputs(input_seq, recd_token, embed_table, rec_embed_table,
                cheb_w1, cheb_b1, cheb_w2, cheb_b2, bl=BL, n_cores=N_CORES):
    seq = np.asarray(input_seq, dtype=np.int64).astype(np.int32)
    tok = np.asarray(recd_token, dtype=np.int64).astype(np.int32)
    embed = np.asarray(embed_table, np.float32).astype(ml_dtypes.bfloat16)
    rec = np.asarray(rec_embed_table, np.float32).astype(ml_dtypes.bfloat16)
    w1 = np.asarray(cheb_w1, dtype=np.float32)
    w2 = np.asarray(cheb_w2, dtype=np.float32)
    # u = [w0 - w2, w1, 2*w2]: Chebyshev t2 = 2*S*t1 - t0 fold
    u1 = np.ascontiguousarray(
        np.stack([w1[0] - w1[2], w1[1], 2.0 * w1[2]], axis=0
                 ).transpose(1, 0, 2)).astype(ml_dtypes.bfloat16)
    u2 = np.ascontiguousarray(
        np.stack([w2[0] - w2[2], w2[1], 2.0 * w2[2]], axis=0
                 ).transpose(1, 0, 2)).astype(ml_dtypes.bfloat16)
    b1 = np.asarray(cheb_b1, dtype=np.float32).reshape(P, 1)
    b2 = np.asarray(cheb_b2, dtype=np.float32).reshape(P, 1)

    pidx = np.arange(P)[:, None, None]
    nbidx = np.arange(NB)[None, :, None]
    cidx = np.arange(N)[None, None, :]
    noeyed = (cidx != nbidx * P + pidx).astype(ml_dtypes.bfloat16)

    maps = []
    for ci in range(n_cores):
        g0 = ci * bl
        idxn = np.ascontiguousarray(
            seq[g0:g0 + bl].reshape(bl, NB, P).transpose(0, 2, 1))
        idxa = np.ascontiguousarray(
            tok[g0:g0 + bl].reshape(bl, KC, P).transpose(0, 2, 1))
        maps.append({
            "idxn": idxn, "idxa": idxa, "embed": embed, "rec": rec,
            "noeyed": noeyed, "u1d": u1, "u2d": u2, "b1d": b1, "b2d": b2,
        })
    return maps


_NC_CACHE = {}


def _get_nc(bl=BL):
    if bl not in _NC_CACHE:
        _NC_CACHE[bl] = build(bl)
    return _NC_CACHE[bl]


def kernel(input_seq, recd_token, embed_table, rec_embed_table,
           cheb_w1, cheb_b1, cheb_w2, cheb_b2):
    nc = _get_nc()
    maps = host_inputs(input_seq, recd_token, embed_table, rec_embed_table,
                       cheb_w1, cheb_b1, cheb_w2, cheb_b2)
    res = run_bass_kernel_spmd(nc, maps, core_ids=list(range(N_CORES)))
    parts = []
    for ci in range(N_CORES):
        r = res.results[ci]
        ox = np.asarray(r["out_x"]).astype(np.float32)
        x = np.ascontiguousarray(ox.transpose(0, 2, 1)).reshape(BL * N, D)
        oa = np.asarray(r["out_app"]).astype(np.float32)
        oa = np.ascontiguousarray(oa.transpose(0, 2, 1, 3)).reshape(BL * N, R)
        parts.append(np.concatenate([x, oa], axis=1))
    return np.concatenate(parts, axis=0)
